# revision 28
# baseline (speedup 1.0000x reference)
"""3-layer GCN (B=32 graphs, N=512 nodes, D=512 feats) on 8 trn2 NeuronCores.

Sharding: data-parallel over graphs — 4 graphs per core, weights replicated.

Math per graph g, per layer l:  h <- adj @ (h @ Wl) + bl  (relu on l=0,1).

Device layout trick: each layer is two matmuls whose contraction dims
alternate (features d, then nodes m).  We chain them with no on-device
transposes by keeping the layer input as G = H^T (feature-on-partition):
  MM1: S[n_i, e]   = sum_d G[d, n_i]^T W[d, e]     (lhsT=G chunk, rhs=W)
  MM2: G'[e_j, n]  = sum_m S[m, e_j]^T A^T[m, n]   (lhsT=S chunk, rhs=A^T)
MM2's output is already H'^T, feeding the next layer's MM1.  The host
pre-transposes batch_graph (-> X^T) and adj (-> A^T) and transposes the
final output back; those are free w.r.t. HW kernel time.

Performance notes (vs the fp32r baseline):
  * All matmul operands are bf16 (same 1 cyc/row PE cost as fp32r, half
    the DMA bytes and SBUF).  PSUM accumulation stays fp32.
  * Graph-0 / W0 loads are chunk-granular and interleaved so the first
    matmul starts ~3us in; every other load is coalesced into one DMA
    per tensor (per-DMA descriptor-gen overhead is the scarce resource,
    not bandwidth).  W1+W2 and all biases ship as single concatenated
    host-side tensors.
  * Output stores ride the ACT HWDGE ring (loads own the SP ring); the
    three non-final graphs store once per graph.
  * The final unit's last output chunk is computed as a 384-col group
    followed by a 128-col group, so only a small bias-add + small store
    (on the otherwise-idle SP ring) trails the last matmul.
"""

import numpy as np
from ml_dtypes import bfloat16

import concourse.mybir as mybir
import concourse.tile as tile
from concourse import bacc
from concourse.bass_utils import run_bass_kernel_spmd

B, N, D = 32, 512, 512
N_CORES = 8
GPC = B // N_CORES  # graphs per core
P = 128
KO = D // P  # 128-partition chunks per 512 dim
MM_DT = mybir.dt.bfloat16

_CACHE = {}
LAST_RESULTS = None


def _build(reps=1):
    f32 = mybir.dt.float32
    nc = bacc.Bacc("TRN2", target_bir_lowering=False, debug=False)

    xt = nc.dram_tensor("xt", [GPC, D, N], MM_DT, kind="ExternalInput").ap()
    at = nc.dram_tensor("at", [GPC, N, N], MM_DT, kind="ExternalInput").ap()
    # Graph-0 and W0 chunks pre-interleaved on the host ([P, KO, {g0,w0}, N])
    # so each startup chunk pair arrives with a single DMA dispatch.  The
    # first matmul's exact needs (g0k0 cols 0:128 + w0k0) are packed into a
    # separate minimal tensor so the PE can start one transfer-time earlier.
    boot0 = nc.dram_tensor("boot0", [P, P + N], MM_DT, kind="ExternalInput").ap()
    boot0b = nc.dram_tensor("boot0b", [P, N - P], MM_DT, kind="ExternalInput").ap()
    boot = nc.dram_tensor("boot", [P, KO - 1, 2, N], MM_DT, kind="ExternalInput").ap()
    # W1 and W2 concatenated; biases pre-tiled to [P, 3*KO] on the host.
    wcat = nc.dram_tensor("wcat", [2, D, D], MM_DT, kind="ExternalInput").ap()
    bcat = nc.dram_tensor("bcat", [P, 3 * KO], f32, kind="ExternalInput").ap()
    out = nc.dram_tensor("out", [GPC, D, N], f32, kind="ExternalOutput").ap()

    relu = mybir.ActivationFunctionType.Relu

    from contextlib import ExitStack

    with tile.TileContext(nc) as tc:
        with (
            tc.tile_pool(name="weights", bufs=1) as wpool,
            tc.tile_pool(name="gbuf", bufs=3) as gpool,
            tc.tile_pool(name="hbuf", bufs=32) as hpool,
            tc.tile_pool(name="adj", bufs=4) as apool,
            tc.tile_pool(name="sbuf_s", bufs=8) as spool,
            tc.tile_pool(name="outp", bufs=2) as opool,
            tc.tile_pool(name="psum", bufs=4, space="PSUM") as pspool,
            ExitStack() as loop_ctx,
        ):
            w12_sb = wpool.tile([P, 2, KO, D], MM_DT, tag="w12", name="w12_sb")
            b_sb = wpool.tile([P, 3, KO], f32, tag="b", name="b_sb")

            wcat_r = wcat.rearrange("l (ko p) e -> p l ko e", p=P)
            xt_r = [xt[g].rearrange("(ko p) n -> p ko n", p=P) for g in range(GPC)]
            at_r = [at[g].rearrange("(ko p) n -> p ko n", p=P) for g in range(GPC)]
            out_r = [out[g].rearrange("(ko p) n -> p ko n", p=P) for g in range(GPC)]

            if reps > 1:
                loop_ctx.enter_context(tc.For_i(0, reps, 1))

            # Layer-input chunk accessors: chunk(k, cols) -> AP.  Graph 0's
            # layer-0 input is 4 separate tiles (fine-grained startup deps);
            # other graphs use one [P, KO, N] tile loaded with a single DMA.
            def chunks_of(t3d):
                return [
                    (lambda cs, _t=t3d, _k=k: _t[:, _k, cs]) for k in range(KO)
                ]

            gts = [None] * GPC
            ats = [None] * GPC

            # Startup: a minimal first DMA (exactly the first matmul's
            # operands), the rest of the g0k0 chunk, then one packed DMA per
            # remaining (g0, w0) chunk pair.
            bt0 = gpool.tile([P, P + N], MM_DT, tag="bt0", name="bt0")
            nc.sync.dma_start(bt0[:], boot0[:, :])
            bt0b = gpool.tile([P, N - P], MM_DT, tag="bt0b", name="bt0b")
            nc.sync.dma_start(bt0b[:], boot0b[:, :])
            bt = [
                gpool.tile([P, 2, N], MM_DT, tag=f"bt{k}", name=f"bt_{k}")
                for k in range(1, KO)
            ]
            for k in range(1, KO):
                nc.sync.dma_start(bt[k - 1][:], boot[:, k - 1, :, :])

            def g0_chunk0(cs):
                # cols 0:128 live in the minimal boot tile; the rest in bt0b
                if cs.stop <= P:
                    return bt0[:, cs]
                return bt0b[:, cs.start - P : cs.stop - P]

            gts[0] = [g0_chunk0] + [
                (lambda cs, _t=bt[k - 1]: _t[:, 0, cs]) for k in range(1, KO)
            ]
            w0_sb = [bt0[:, P : P + N]] + [bt[k - 1][:, 1, :] for k in range(1, KO)]

            a0 = apool.tile([P, KO, N], MM_DT, tag="a", name="a0")
            nc.sync.dma_start(a0[:], at_r[0][:, :, :])
            ats[0] = a0
            nc.sync.dma_start(b_sb[:], bcat.rearrange("p (l ko) -> p l ko", ko=KO))
            for g in range(1, GPC):
                gt = gpool.tile([P, KO, N], MM_DT, tag="gx", name=f"g{g}")
                nc.sync.dma_start(gt[:], xt_r[g][:, :, :])
                gts[g] = chunks_of(gt)
                a_t = apool.tile([P, KO, N], MM_DT, tag="a", name=f"a{g}")
                nc.sync.dma_start(a_t[:], at_r[g][:, :, :])
                ats[g] = a_t
            nc.sync.dma_start(w12_sb[:], wcat_r[:, :, :, :])

            def w_chunk(l, k):
                return w0_sb[k] if l == 0 else w12_sb[:, l - 1, k, :]

            for l in range(3):
                last = l == 2
                for g in range(GPC):
                    gt, a_t = gts[g], ats[g]
                    final_unit = last and g == GPC - 1

                    # MM1: S[n_i, :] = sum_k G_k[:, n_i].T @ W_k
                    s_t = [None] * KO
                    for i in range(KO):
                        ps = pspool.tile([P, D], f32, tag="ps")
                        for k in range(KO):
                            nc.tensor.matmul(
                                ps[:],
                                lhsT=gt[k](slice(P * i, P * (i + 1))),
                                rhs=w_chunk(l, k),
                                start=(k == 0),
                                stop=(k == KO - 1),
                            )
                        s_t[i] = spool.tile([P, D], MM_DT, tag="s", name=f"s_{i}")
                        nc.vector.tensor_copy(s_t[i][:], ps[:])

                    # MM2: G'[e_j, :] = sum_k S_k[:, e_j].T @ A^T_k
                    # The very last unit computes its final output chunk in
                    # two pieces with the big piece hoisted early, so only a
                    # small [P,128] bias+store chain trails the last matmul.
                    if final_unit:
                        jn = N - P
                        groups = [
                            (0, slice(0, N)),
                            (1, slice(0, N)),
                            (2, slice(0, N)),
                            (KO - 1, slice(0, jn)),
                            (KO - 1, slice(jn, N)),
                        ]
                    else:
                        groups = [(j, slice(0, N)) for j in range(KO)]
                    if last and not final_unit:
                        o_full = opool.tile([P, KO, N], f32, tag="o", name=f"o{g}")
                    nxt = [None] * KO
                    for j, cs in groups:
                        fw = cs.stop - cs.start
                        pz = pspool.tile([P, fw], f32, tag="pz")
                        for k in range(KO):
                            nc.tensor.matmul(
                                pz[:],
                                lhsT=s_t[k][:, P * j : P * (j + 1)],
                                rhs=a_t[:, k, cs],
                                start=(k == 0),
                                stop=(k == KO - 1),
                            )
                        if final_unit:
                            o_t = opool.tile(
                                [P, fw], f32, tag=f"of{fw}", name=f"of{j}_{cs.start}"
                            )
                            nc.vector.tensor_scalar_add(
                                o_t[:], pz[:], b_sb[:, l, j : j + 1]
                            )
                            # The trailing stores ride the idle SP ring.
                            eng = nc.sync if fw != N else nc.scalar
                            eng.dma_start(out_r[g][:, j, cs], o_t[:])
                        elif last:
                            nc.vector.tensor_scalar_add(
                                o_full[:, j, :], pz[:], b_sb[:, l, j : j + 1]
                            )
                        else:
                            gn = hpool.tile([P, fw], MM_DT, tag="g", name=f"gn{j}")
                            nc.scalar.activation(
                                gn[:], pz[:], relu, bias=b_sb[:, l, j : j + 1]
                            )
                            nxt[j] = gn
                    if last and not final_unit:
                        nc.scalar.dma_start(out_r[g][:, :, :], o_full[:])
                    if not last:
                        gts[g] = [
                            (lambda cs, _t=nxt[k]: _t[:, cs]) for k in range(KO)
                        ]

    nc.compile()
    return nc


def _host_prep(batch_graph, adj, W0, b0, W1, b1, W2, b2):
    """Cast to bf16 / transpose / concatenate on host; build per-core maps."""
    xt = np.ascontiguousarray(
        np.asarray(batch_graph, np.float32).transpose(0, 2, 1)
    ).astype(bfloat16)
    at = np.ascontiguousarray(
        np.asarray(adj, np.float32).transpose(0, 2, 1)
    ).astype(bfloat16)
    w0 = np.asarray(W0, np.float32).astype(bfloat16)
    wcat = np.stack(
        [np.asarray(W1, np.float32), np.asarray(W2, np.float32)]
    ).astype(bfloat16)
    # bcat[p, l*KO + ko] = b_l[ko*P + p]
    bs = np.stack([np.asarray(b, np.float32) for b in (b0, b1, b2)])  # [3, D]
    bcat = np.ascontiguousarray(
        bs.reshape(3, KO, P).transpose(2, 0, 1).reshape(P, 3 * KO)
    )
    w0_pko = w0.reshape(KO, P, D).transpose(1, 0, 2)  # [P, KO, D]

    in_maps = []
    for c in range(N_CORES):
        sl = slice(c * GPC, (c + 1) * GPC)
        # boot[p, k-1, 0, :] = core's graph-0 X^T chunk k; [.., 1, :] = W0.
        g0_pko = xt[c * GPC].reshape(KO, P, N).transpose(1, 0, 2)  # [P, KO, N]
        bootc = np.ascontiguousarray(
            np.stack([g0_pko[:, 1:], w0_pko[:, 1:]], axis=2)  # [P, KO-1, 2, N]
        )
        boot0c = np.ascontiguousarray(
            np.concatenate([g0_pko[:, 0, :P], w0_pko[:, 0, :]], axis=1)
        )
        boot0bc = np.ascontiguousarray(g0_pko[:, 0, P:])
        in_maps.append(
            {
                "xt": np.ascontiguousarray(xt[sl]),
                "at": np.ascontiguousarray(at[sl]),
                "boot0": boot0c,
                "boot0b": boot0bc,
                "boot": bootc,
                "wcat": wcat,
                "bcat": bcat,
            }
        )
    return in_maps


def kernel(batch_graph, adj, W0, b0, W1, b1, W2, b2, trace=False):
    global LAST_RESULTS
    if "nc" not in _CACHE:
        _CACHE["nc"] = _build()
    nc = _CACHE["nc"]

    in_maps = _host_prep(batch_graph, adj, W0, b0, W1, b1, W2, b2)

    try:
        res = run_bass_kernel_spmd(
            nc, in_maps, core_ids=list(range(N_CORES)), trace=trace
        )
    except ModuleNotFoundError:
        # Tracing was requested (arg or BASS_TRACE env) but this environment
        # lacks the axon NTFF profile hook; rerun without the trace path.
        import os

        os.environ["BASS_NEVER_TRACE"] = "1"
        try:
            res = run_bass_kernel_spmd(
                nc, in_maps, core_ids=list(range(N_CORES)), trace=False
            )
        finally:
            del os.environ["BASS_NEVER_TRACE"]
    LAST_RESULTS = res
    outs = [r["out"].transpose(0, 2, 1) for r in res.results]  # [GPC, N, D] each
    return np.ascontiguousarray(np.concatenate(outs, axis=0), dtype=np.float32)


# revision 37
# speedup vs baseline: 1.2241x; 1.2241x over previous
"""3-layer GCN (B=32 graphs, N=512 nodes, D=512 feats) on 8 trn2 NeuronCores.

Sharding: data-parallel over graphs — 4 graphs per core, weights replicated.

Math per graph g, per layer l:  h <- adj @ (h @ Wl) + bl  (relu on l=0,1).

Device layout trick: each layer is two matmuls whose contraction dims
alternate (features d, then nodes m).  We chain them with no on-device
transposes by keeping the layer input as G = H^T (feature-on-partition):
  MM1: S[n_i, e]   = sum_d G[d, n_i]^T W[d, e]     (lhsT=G chunk, rhs=W)
  MM2: G'[e_j, n]  = sum_m S[m, e_j]^T A^T[m, n]   (lhsT=S chunk, rhs=A^T)
MM2's output is already H'^T, feeding the next layer's MM1.  The host
pre-transposes batch_graph (-> X^T) and adj (-> A^T) and transposes the
final output back; those are free w.r.t. HW kernel time.

Performance notes (vs the fp32r baseline):
  * All matmul operands are bf16 (same 1 cyc/row PE cost as fp32r, half
    the DMA bytes and SBUF).  PSUM accumulation stays fp32.
  * Graph-0 / W0 loads are chunk-granular and interleaved so the first
    matmul starts ~3us in; every other load is coalesced into one DMA
    per tensor (per-DMA descriptor-gen overhead is the scarce resource,
    not bandwidth).  W1+W2 and all biases ship as single concatenated
    host-side tensors.
  * Output stores ride the ACT HWDGE ring (loads own the SP ring); the
    three non-final graphs store once per graph.
  * The final unit's last output chunk is computed as a 384-col group
    followed by a 128-col group, so only a small bias-add + small store
    (on the otherwise-idle SP ring) trails the last matmul.
"""

import numpy as np
from ml_dtypes import bfloat16

import concourse.mybir as mybir
import concourse.tile as tile
from concourse import bacc
from concourse.bass_utils import run_bass_kernel_spmd

B, N, D = 32, 512, 512
N_CORES = 8
GPC = B // N_CORES  # graphs per core
P = 128
KO = D // P  # 128-partition chunks per 512 dim
MM_DT = mybir.dt.bfloat16

_CACHE = {}
LAST_RESULTS = None


def _build(reps=1):
    f32 = mybir.dt.float32
    nc = bacc.Bacc("TRN2", target_bir_lowering=False, debug=False)

    xt = nc.dram_tensor("xt", [GPC, D, N], MM_DT, kind="ExternalInput").ap()
    at = nc.dram_tensor("at", [GPC, N, N], MM_DT, kind="ExternalInput").ap()
    # Graph-0 and W0 chunks pre-interleaved on the host ([P, KO, {g0,w0}, N])
    # so each startup chunk pair arrives with a single DMA dispatch.  The
    # first matmul's exact needs (g0k0 cols 0:128 + w0k0) are packed into a
    # separate minimal tensor so the PE can start one transfer-time earlier.
    boot0 = nc.dram_tensor("boot0", [P, P + N], MM_DT, kind="ExternalInput").ap()
    boot0b = nc.dram_tensor("boot0b", [P, N - P], MM_DT, kind="ExternalInput").ap()
    boot = nc.dram_tensor("boot", [P, KO - 1, 2, N], MM_DT, kind="ExternalInput").ap()
    # W1 and W2 concatenated; biases pre-tiled to [P, 3*KO] on the host.
    wcat = nc.dram_tensor("wcat", [2, D, D], MM_DT, kind="ExternalInput").ap()
    bcat = nc.dram_tensor("bcat", [P, 3 * KO], f32, kind="ExternalInput").ap()
    out = nc.dram_tensor("out", [GPC, D, N], f32, kind="ExternalOutput").ap()

    relu = mybir.ActivationFunctionType.Relu

    from contextlib import ExitStack

    with tile.TileContext(nc) as tc:
        with (
            tc.tile_pool(name="weights", bufs=1) as wpool,
            tc.tile_pool(name="gbuf", bufs=3) as gpool,
            tc.tile_pool(name="hbuf", bufs=32) as hpool,
            tc.tile_pool(name="adj", bufs=4) as apool,
            tc.tile_pool(name="sbuf_s", bufs=8) as spool,
            tc.tile_pool(name="outp", bufs=2) as opool,
            tc.tile_pool(name="psum", bufs=4, space="PSUM") as pspool,
            ExitStack() as loop_ctx,
        ):
            w12_sb = wpool.tile([P, 2, KO, D], MM_DT, tag="w12", name="w12_sb")
            b_sb = wpool.tile([P, 3, KO], f32, tag="b", name="b_sb")

            wcat_r = wcat.rearrange("l (ko p) e -> p l ko e", p=P)
            xt_r = [xt[g].rearrange("(ko p) n -> p ko n", p=P) for g in range(GPC)]
            at_r = [at[g].rearrange("(ko p) n -> p ko n", p=P) for g in range(GPC)]
            out_r = [out[g].rearrange("(ko p) n -> p ko n", p=P) for g in range(GPC)]

            if reps > 1:
                loop_ctx.enter_context(tc.For_i(0, reps, 1))

            # Layer-input chunk accessors: chunk(k, cols) -> AP.  Graph 0's
            # layer-0 input is 4 separate tiles (fine-grained startup deps);
            # other graphs use one [P, KO, N] tile loaded with a single DMA.
            def chunks_of(t3d):
                return [
                    (lambda cs, _t=t3d, _k=k: _t[:, _k, cs]) for k in range(KO)
                ]

            gts = [None] * GPC
            ats = [None] * GPC

            # Startup: a minimal first DMA (exactly the first matmul's
            # operands), the rest of the g0k0 chunk, then one packed DMA per
            # remaining (g0, w0) chunk pair.
            bt0 = gpool.tile([P, P + N], MM_DT, tag="bt0", name="bt0")
            nc.sync.dma_start(bt0[:], boot0[:, :])
            bt0b = gpool.tile([P, N - P], MM_DT, tag="bt0b", name="bt0b")
            nc.sync.dma_start(bt0b[:], boot0b[:, :])
            bt = [
                gpool.tile([P, 2, N], MM_DT, tag=f"bt{k}", name=f"bt_{k}")
                for k in range(1, KO)
            ]
            for k in range(1, KO):
                nc.sync.dma_start(bt[k - 1][:], boot[:, k - 1, :, :])

            def g0_chunk0(cs):
                # cols 0:128 live in the minimal boot tile; the rest in bt0b
                if cs.stop <= P:
                    return bt0[:, cs]
                return bt0b[:, cs.start - P : cs.stop - P]

            gts[0] = [g0_chunk0] + [
                (lambda cs, _t=bt[k - 1]: _t[:, 0, cs]) for k in range(1, KO)
            ]
            w0_sb = [bt0[:, P : P + N]] + [bt[k - 1][:, 1, :] for k in range(1, KO)]

            a0 = apool.tile([P, KO, N], MM_DT, tag="a", name="a0")
            nc.sync.dma_start(a0[:], at_r[0][:, :, :])
            ats[0] = a0
            nc.sync.dma_start(b_sb[:], bcat.rearrange("p (l ko) -> p l ko", ko=KO))
            for g in range(1, GPC):
                gt = gpool.tile([P, KO, N], MM_DT, tag="gx", name=f"g{g}")
                nc.sync.dma_start(gt[:], xt_r[g][:, :, :])
                gts[g] = chunks_of(gt)
                a_t = apool.tile([P, KO, N], MM_DT, tag="a", name=f"a{g}")
                nc.sync.dma_start(a_t[:], at_r[g][:, :, :])
                ats[g] = a_t
            nc.sync.dma_start(w12_sb[:], wcat_r[:, :, :, :])

            def w_chunk(l, k):
                return w0_sb[k] if l == 0 else w12_sb[:, l - 1, k, :]

            for l in range(3):
                last = l == 2
                for g in range(GPC):
                    gt, a_t = gts[g], ats[g]
                    final_unit = last and g == GPC - 1

                    # MM1: S[n_i, :] = sum_k G_k[:, n_i].T @ W_k
                    # The very first unit is emitted k-outer so the PE stream
                    # consumes startup chunk pairs exactly in arrival order.
                    s_t = [None] * KO
                    if l == 0 and g == 0:
                        pss = []
                        for i in range(KO):
                            pss.append(pspool.tile([P, D], f32, tag="ps", name=f"ps{i}"))
                        for k in range(KO):
                            for i in range(KO):
                                nc.tensor.matmul(
                                    pss[i][:],
                                    lhsT=gt[k](slice(P * i, P * (i + 1))),
                                    rhs=w_chunk(l, k),
                                    start=(k == 0),
                                    stop=(k == KO - 1),
                                )
                        for i in range(KO):
                            s_t[i] = spool.tile(
                                [P, D], MM_DT, tag="s", name=f"s_{i}"
                            )
                            nc.vector.tensor_copy(s_t[i][:], pss[i][:])
                    else:
                        for i in range(KO):
                            ps = pspool.tile([P, D], f32, tag="ps")
                            for k in range(KO):
                                nc.tensor.matmul(
                                    ps[:],
                                    lhsT=gt[k](slice(P * i, P * (i + 1))),
                                    rhs=w_chunk(l, k),
                                    start=(k == 0),
                                    stop=(k == KO - 1),
                                )
                            s_t[i] = spool.tile(
                                [P, D], MM_DT, tag="s", name=f"s_{i}"
                            )
                            nc.vector.tensor_copy(s_t[i][:], ps[:])

                    # MM2: G'[e_j, :] = sum_k S_k[:, e_j].T @ A^T_k
                    # The very last unit computes its final output chunk in
                    # two pieces with the big piece hoisted early, so only a
                    # small [P,128] bias+store chain trails the last matmul.
                    if final_unit:
                        jn = N - P
                        groups = [
                            (0, slice(0, N)),
                            (1, slice(0, N)),
                            (2, slice(0, N)),
                            (KO - 1, slice(0, jn)),
                            (KO - 1, slice(jn, N)),
                        ]
                    else:
                        groups = [(j, slice(0, N)) for j in range(KO)]
                    if last and not final_unit:
                        o_full = opool.tile([P, KO, N], f32, tag="o", name=f"o{g}")
                    nxt = [None] * KO
                    for j, cs in groups:
                        fw = cs.stop - cs.start
                        pz = pspool.tile([P, fw], f32, tag="pz")
                        for k in range(KO):
                            nc.tensor.matmul(
                                pz[:],
                                lhsT=s_t[k][:, P * j : P * (j + 1)],
                                rhs=a_t[:, k, cs],
                                start=(k == 0),
                                stop=(k == KO - 1),
                            )
                        if final_unit:
                            o_t = opool.tile(
                                [P, fw], f32, tag=f"of{fw}", name=f"of{j}_{cs.start}"
                            )
                            nc.vector.tensor_scalar_add(
                                o_t[:], pz[:], b_sb[:, l, j : j + 1]
                            )
                            # All final-unit stores ride the idle SP ring.
                            nc.sync.dma_start(out_r[g][:, j, cs], o_t[:])
                        elif last:
                            nc.vector.tensor_scalar_add(
                                o_full[:, j, :], pz[:], b_sb[:, l, j : j + 1]
                            )
                        else:
                            gn = hpool.tile([P, fw], MM_DT, tag="g", name=f"gn{j}")
                            nc.scalar.activation(
                                gn[:], pz[:], relu, bias=b_sb[:, l, j : j + 1]
                            )
                            nxt[j] = gn
                    if last and not final_unit:
                        nc.scalar.dma_start(out_r[g][:, :, :], o_full[:])
                    if not last:
                        gts[g] = [
                            (lambda cs, _t=nxt[k]: _t[:, cs]) for k in range(KO)
                        ]

    nc.compile()
    return nc


def _host_prep(batch_graph, adj, W0, b0, W1, b1, W2, b2):
    """Cast to bf16 / transpose / concatenate on host; build per-core maps."""
    xt = np.ascontiguousarray(
        np.asarray(batch_graph, np.float32).transpose(0, 2, 1)
    ).astype(bfloat16)
    at = np.ascontiguousarray(
        np.asarray(adj, np.float32).transpose(0, 2, 1)
    ).astype(bfloat16)
    w0 = np.asarray(W0, np.float32).astype(bfloat16)
    wcat = np.stack(
        [np.asarray(W1, np.float32), np.asarray(W2, np.float32)]
    ).astype(bfloat16)
    # bcat[p, l*KO + ko] = b_l[ko*P + p]
    bs = np.stack([np.asarray(b, np.float32) for b in (b0, b1, b2)])  # [3, D]
    bcat = np.ascontiguousarray(
        bs.reshape(3, KO, P).transpose(2, 0, 1).reshape(P, 3 * KO)
    )
    w0_pko = w0.reshape(KO, P, D).transpose(1, 0, 2)  # [P, KO, D]

    in_maps = []
    for c in range(N_CORES):
        sl = slice(c * GPC, (c + 1) * GPC)
        # boot[p, k-1, 0, :] = core's graph-0 X^T chunk k; [.., 1, :] = W0.
        g0_pko = xt[c * GPC].reshape(KO, P, N).transpose(1, 0, 2)  # [P, KO, N]
        bootc = np.ascontiguousarray(
            np.stack([g0_pko[:, 1:], w0_pko[:, 1:]], axis=2)  # [P, KO-1, 2, N]
        )
        boot0c = np.ascontiguousarray(
            np.concatenate([g0_pko[:, 0, :P], w0_pko[:, 0, :]], axis=1)
        )
        boot0bc = np.ascontiguousarray(g0_pko[:, 0, P:])
        in_maps.append(
            {
                "xt": np.ascontiguousarray(xt[sl]),
                "at": np.ascontiguousarray(at[sl]),
                "boot0": boot0c,
                "boot0b": boot0bc,
                "boot": bootc,
                "wcat": wcat,
                "bcat": bcat,
            }
        )
    return in_maps


def kernel(batch_graph, adj, W0, b0, W1, b1, W2, b2, trace=False):
    global LAST_RESULTS
    if "nc" not in _CACHE:
        _CACHE["nc"] = _build()
    nc = _CACHE["nc"]

    in_maps = _host_prep(batch_graph, adj, W0, b0, W1, b1, W2, b2)

    try:
        res = run_bass_kernel_spmd(
            nc, in_maps, core_ids=list(range(N_CORES)), trace=trace
        )
    except ModuleNotFoundError:
        # Tracing was requested (arg or BASS_TRACE env) but this environment
        # lacks the axon NTFF profile hook; rerun without the trace path.
        import os

        os.environ["BASS_NEVER_TRACE"] = "1"
        try:
            res = run_bass_kernel_spmd(
                nc, in_maps, core_ids=list(range(N_CORES)), trace=False
            )
        finally:
            del os.environ["BASS_NEVER_TRACE"]
    LAST_RESULTS = res
    outs = [r["out"].transpose(0, 2, 1) for r in res.results]  # [GPC, N, D] each
    return np.ascontiguousarray(np.concatenate(outs, axis=0), dtype=np.float32)
